# revision 28
# baseline (speedup 1.0000x reference)
"""DeepseekV3 MoE (E=16, K=4, H=1024, I=512, shared 2x) on 8 trn2 NeuronCores.

Expert-parallel: 2 routed experts per core (host gathers each expert's tokens),
shared expert + residual data-parallel over 512-token slices. Host does the
gate (fp32 numpy, reference-exact) and the token all-to-all (gather/scatter);
all matmuls/activations run on-device in bf16 with fp32 accumulation.
"""

import os
import sys
import types
import numpy as np
import ml_dtypes

import concourse.bass as bass
import concourse.mybir as mybir
import concourse.tile as tile
from concourse import bacc
from concourse.bass_utils import run_bass_kernel_spmd

BF16 = mybir.dt.bfloat16
F32 = mybir.dt.float32
NP_BF16 = ml_dtypes.bfloat16

E, K, NG, TG = 16, 4, 4, 2
SCALE = 2.5
H, I, SH_I = 1024, 512, 1024
B, S = 2, 2048
N = B * S
NCORES = 8
EPC = E // NCORES          # experts per core = 2
NSH = N // NCORES          # shared-expert tokens per core = 512
HC = H // 128              # 8 h-chunks
IC = I // 128              # 4 i-chunks (routed)
SIC = SH_I // 128          # 8 i-chunks (shared)
MIN_CE = 1152              # per-expert token capacity (multiple of 128)


def _gate_cw(xf: np.ndarray, gate_w: np.ndarray, gate_bias: np.ndarray) -> np.ndarray:
    """Reference-exact MoE gate in numpy fp32. Returns cw [N, E]."""
    logits = xf @ gate_w.T
    scores = 1.0 / (1.0 + np.exp(-logits))
    sfc = scores + gate_bias
    epg = E // NG
    grp = sfc.reshape(N, NG, epg)
    top2 = np.sort(grp, axis=-1)[:, :, -2:].sum(-1)
    gidx = np.argsort(-top2, axis=1, kind="stable")[:, :TG]
    gmask = np.zeros((N, NG), bool)
    np.put_along_axis(gmask, gidx, True, axis=1)
    emask = np.repeat(gmask, epg, axis=1)
    masked = np.where(emask, sfc, -np.inf)
    topk_idx = np.argsort(-masked, axis=1, kind="stable")[:, :K]
    topk_w = np.take_along_axis(scores, topk_idx, axis=1)
    topk_w = topk_w / (topk_w.sum(-1, keepdims=True) + 1e-20)
    topk_w = topk_w * SCALE
    cw = np.zeros((N, E), np.float32)
    np.put_along_axis(cw, topk_idx, topk_w.astype(np.float32), axis=1)
    return cw


_BUILD_CACHE: dict[int, object] = {}


def _build(cea: int, ceb: int):
    """Build + compile the per-core SPMD Tile program.

    Slot A holds the heavier expert (capacity cea), slot B the lighter
    one (ceb); host pairs experts so every core sees the same shapes.
    """
    key = (cea, ceb)
    if key in _BUILD_CACHE:
        return _BUILD_CACHE[key]
    slots = [(0, cea), (cea, ceb)]   # (col base, capacity)
    tt_total = (cea + ceb) // 128    # routed token tiles per core
    m = cea + ceb                    # routed token slots per core

    nc = bacc.Bacc("TRN2", target_bir_lowering=False, debug=False,
                   num_devices=NCORES)
    xg_t = nc.dram_tensor("xg_t", [H, m], BF16, kind="ExternalInput").ap()
    cw_pt = nc.dram_tensor("cw_pt", [128, tt_total], F32, kind="ExternalInput").ap()
    wg_t = nc.dram_tensor("wg_t", [EPC, H, I], BF16, kind="ExternalInput").ap()
    wu_t = nc.dram_tensor("wu_t", [EPC, H, I], BF16, kind="ExternalInput").ap()
    wd_t = nc.dram_tensor("wd_t", [EPC, I, H], BF16, kind="ExternalInput").ap()
    xs_t = nc.dram_tensor("xs_t", [H, NSH], BF16, kind="ExternalInput").ap()
    xres = nc.dram_tensor("xres", [NSH, H], F32, kind="ExternalInput").ap()
    wsg_t = nc.dram_tensor("wsg_t", [H, SH_I], BF16, kind="ExternalInput").ap()
    wsu_t = nc.dram_tensor("wsu_t", [H, SH_I], BF16, kind="ExternalInput").ap()
    wsd_t = nc.dram_tensor("wsd_t", [SH_I, H], BF16, kind="ExternalInput").ap()
    ident_in = nc.dram_tensor("ident_in", [128, 128], BF16,
                              kind="ExternalInput").ap()
    yg = nc.dram_tensor("yg", [m, H], BF16, kind="ExternalOutput").ap()
    ybase = nc.dram_tensor("ybase", [NSH, H], F32, kind="ExternalOutput").ap()

    MULT = mybir.AluOpType.mult
    SIGM = mybir.ActivationFunctionType.Sigmoid

    with tile.TileContext(nc) as tc:
        with (
            tc.tile_pool(name="const", bufs=1) as const,
            tc.tile_pool(name="sb_act", bufs=3) as sb_act,
            tc.tile_pool(name="sb_out", bufs=3) as sb_out,
            tc.tile_pool(name="ps_gu", bufs=4, space=bass.MemorySpace.PSUM) as ps_gu,
            tc.tile_pool(name="ps_tp", bufs=2, space=bass.MemorySpace.PSUM) as ps_tp,
            tc.tile_pool(name="ps_y", bufs=2, space=bass.MemorySpace.PSUM) as ps_y,
        ):
            ident = const.tile([128, 128], BF16, tag="ident")
            nc.sync.dma_start(ident[:], ident_in[:])

            # resident SBUF loads, ordered so shared-expert compute can
            # start early while routed tokens/weights stream in (Tile
            # tracks DMA->compute deps by SBUF byte bounding box, so each
            # piece lands in a contiguous range).
            xs_sb = const.tile([128, HC, NSH], BF16, tag="xs")
            xs_r = xs_t.rearrange("(c p) m -> p c m", p=128)
            for ch in range(2):
                nc.sync.dma_start(xs_sb[:, ch * 4:(ch + 1) * 4, :],
                                  xs_r[:, ch * 4:(ch + 1) * 4, :])
            wsg_sb = const.tile([128, HC, SH_I], BF16, tag="wsg")
            wsu_sb = const.tile([128, HC, SH_I], BF16, tag="wsu")
            wsd_sb = const.tile([128, SIC, H], BF16, tag="wsd")
            wsg_r = wsg_t.rearrange("(c p) i -> p c i", p=128)
            wsu_r = wsu_t.rearrange("(c p) i -> p c i", p=128)
            for c in range(HC):
                nc.sync.dma_start(wsg_sb[:, c], wsg_r[:, c])
                nc.sync.dma_start(wsu_sb[:, c], wsu_r[:, c])
            nc.sync.dma_start(wsd_sb[:], wsd_t.rearrange("(c p) h -> p c h", p=128))
            cw_sb = const.tile([128, tt_total], F32, tag="cw")
            nc.sync.dma_start(cw_sb[:], cw_pt[:])
            wg_sb = const.tile([128, EPC, HC, I], BF16, tag="wg")
            wu_sb = const.tile([128, EPC, HC, I], BF16, tag="wu")
            wd_sb = const.tile([128, EPC, IC, H], BF16, tag="wd")
            wg_r = wg_t.rearrange("e (c p) i -> p e c i", p=128)
            wu_r = wu_t.rearrange("e (c p) i -> p e c i", p=128)
            wd_r = wd_t.rearrange("e (c p) h -> p e c h", p=128)
            xg_sb = const.tile([128, HC, m], BF16, tag="xg")
            xg_r = xg_t.rearrange("(c p) m -> p c m", p=128)
            nc.sync.dma_start(wg_sb[:, 0], wg_r[:, 0])
            nc.sync.dma_start(wu_sb[:, 0], wu_r[:, 0])
            nblk = 4
            bw = m // nblk
            for b in range(nblk):
                wd = m - b * bw if b == nblk - 1 else bw
                nc.sync.dma_start(xg_sb[:, :, b * bw:b * bw + wd],
                                  xg_r[:, :, b * bw:b * bw + wd])
            nc.sync.dma_start(wd_sb[:, 0], wd_r[:, 0])
            nc.sync.dma_start(wg_sb[:, 1], wg_r[:, 1])
            nc.sync.dma_start(wu_sb[:, 1], wu_r[:, 1])
            nc.sync.dma_start(wd_sb[:, 1], wd_r[:, 1])

            # ---- shared expert + residual ----
            for t in range(NSH // 128):
                col = t * 128
                xres_sb = sb_out.tile([128, H], F32, tag="xres")
                nc.sync.dma_start(xres_sb[:], xres[col:col + 128, :])
                act2_t = sb_act.tile([128, SIC * 128], BF16, tag="act_t")
                for ih in range(2):
                    g2 = ps_gu.tile([128, 512], F32, tag="gu")
                    u2 = ps_gu.tile([128, 512], F32, tag="gu")
                    for c in range(HC):
                        nc.tensor.matmul(g2[:], xs_sb[:, c, col:col + 128],
                                         wsg_sb[:, c, ih * 512:(ih + 1) * 512],
                                         start=(c == 0), stop=(c == HC - 1))
                    for c in range(HC):
                        nc.tensor.matmul(u2[:], xs_sb[:, c, col:col + 128],
                                         wsu_sb[:, c, ih * 512:(ih + 1) * 512],
                                         start=(c == 0), stop=(c == HC - 1))
                    sig2 = sb_act.tile([128, I], BF16, tag="sig")
                    nc.scalar.activation(sig2[:], g2[:], SIGM)
                    a2 = sb_act.tile([128, I], BF16, tag="a")
                    nc.vector.tensor_mul(a2[:], sig2[:], g2[:])
                    act2 = sb_act.tile([128, I], BF16, tag="actw")
                    nc.vector.tensor_mul(act2[:], a2[:], u2[:])
                    tp2 = ps_tp.tile([128, I], BF16, tag="tp")
                    for q in range(IC):
                        nc.tensor.transpose(tp2[:, q * 128:(q + 1) * 128],
                                            act2[:, q * 128:(q + 1) * 128],
                                            ident[:])
                    nc.scalar.copy(act2_t[:, ih * 512:(ih + 1) * 512], tp2[:])
                ob_sb = sb_out.tile([128, H], F32, tag="ob")
                for hh in range(2):
                    y2_ps = ps_y.tile([128, 512], F32, tag="y_ps")
                    for qq in range(SIC):
                        nc.tensor.matmul(
                            y2_ps[:], act2_t[:, qq * 128:(qq + 1) * 128],
                            wsd_sb[:, qq, hh * 512:(hh + 1) * 512],
                            start=(qq == 0), stop=(qq == SIC - 1))
                    nc.vector.tensor_add(ob_sb[:, hh * 512:(hh + 1) * 512],
                                         y2_ps[:],
                                         xres_sb[:, hh * 512:(hh + 1) * 512])
                for r in range(4):
                    nc.sync.dma_start(ybase[col + 32 * r:col + 32 * (r + 1), :],
                                      ob_sb[32 * r:32 * (r + 1), :])

            # ---- routed experts (2-stage software pipeline: emit tile
            # t+1's G/U before tile t's act/transpose/down so the PE has
            # fill work during tile t's DVE latency) ----
            def routed_stage_b(e, col, tt, g_ps, u_ps):
                sig_sb = sb_act.tile([128, I], BF16, tag="sig")
                nc.scalar.activation(sig_sb[:], g_ps[:], SIGM)
                a_sb = sb_act.tile([128, I], BF16, tag="a")
                nc.vector.scalar_tensor_tensor(
                    a_sb[:], sig_sb[:], cw_sb[:, tt:tt + 1], g_ps[:],
                    op0=MULT, op1=MULT)
                actw_sb = sb_act.tile([128, I], BF16, tag="actw")
                nc.vector.tensor_mul(actw_sb[:], a_sb[:], u_ps[:])
                tp_ps = ps_tp.tile([128, I], BF16, tag="tp")
                for q in range(IC):
                    nc.tensor.transpose(tp_ps[:, q * 128:(q + 1) * 128],
                                        actw_sb[:, q * 128:(q + 1) * 128],
                                        ident[:])
                act_t = sb_act.tile([128, SIC * 128], BF16, tag="act_t")
                nc.scalar.copy(act_t[:, :I], tp_ps[:])
                y_sb = sb_out.tile([128, H], BF16, tag="y")
                for hh in range(2):
                    y_ps = ps_y.tile([128, 512], F32, tag="y_ps")
                    for q in range(IC):
                        nc.tensor.matmul(
                            y_ps[:], act_t[:, q * 128:(q + 1) * 128],
                            wd_sb[:, e, q, hh * 512:(hh + 1) * 512],
                            start=(q == 0), stop=(q == IC - 1))
                    if hh == 0:
                        nc.scalar.copy(y_sb[:, :512], y_ps[:])
                    else:
                        nc.vector.tensor_copy(y_sb[:, 512:], y_ps[:])
                for r in range(4):
                    nc.sync.dma_start(
                        yg[col + 32 * r:col + 32 * (r + 1), :],
                        y_sb[32 * r:32 * (r + 1), :])

            pend = None
            ttbase = 0
            for e, (base, cap) in enumerate(slots):
                for t in range(cap // 128):
                    col = base + t * 128
                    tt = ttbase + t
                    g_ps = ps_gu.tile([128, I], F32, tag="gu")
                    u_ps = ps_gu.tile([128, I], F32, tag="gu")
                    for c in range(HC):
                        nc.tensor.matmul(g_ps[:], xg_sb[:, c, col:col + 128],
                                         wg_sb[:, e, c, :],
                                         start=(c == 0), stop=(c == HC - 1))
                    for c in range(HC):
                        nc.tensor.matmul(u_ps[:], xg_sb[:, c, col:col + 128],
                                         wu_sb[:, e, c, :],
                                         start=(c == 0), stop=(c == HC - 1))
                    if pend is not None:
                        routed_stage_b(*pend)
                    pend = (e, col, tt, g_ps, u_ps)
                ttbase += cap // 128
            routed_stage_b(*pend)

    nc.compile()
    _BUILD_CACHE[key] = nc
    return nc


def _prepare(inputs: dict, caps, pairs, cw: np.ndarray, idx: list[np.ndarray]):
    """Build per-core input maps. idx[e] = token indices routed to expert e."""
    xf = np.asarray(inputs["hidden_states"], np.float32).reshape(N, H)
    xt_bf = np.ascontiguousarray(xf.T).astype(NP_BF16)        # [H, N]
    wg = np.asarray(inputs["Wg"])
    wu = np.asarray(inputs["Wu"])
    wd = np.asarray(inputs["Wd"])
    wsg = np.asarray(inputs["Ws_g"])
    wsu = np.asarray(inputs["Ws_u"])
    wsd = np.asarray(inputs["Ws_d"])
    bases = [0, caps[0]]
    m = caps[0] + caps[1]
    wsg_bf = np.ascontiguousarray(wsg.T).astype(NP_BF16)
    wsu_bf = np.ascontiguousarray(wsu.T).astype(NP_BF16)
    wsd_bf = np.ascontiguousarray(wsd.T).astype(NP_BF16)
    in_maps = []
    for core in range(NCORES):
        es = pairs[core]
        xg = np.zeros((H, m), NP_BF16)
        cwg = np.zeros((m,), np.float32)
        for j, e in enumerate(es):
            ne = len(idx[e])
            xg[:, bases[j]:bases[j] + ne] = xt_bf[:, idx[e]]
            cwg[bases[j]:bases[j] + ne] = cw[idx[e], e]
        sl = slice(core * NSH, (core + 1) * NSH)
        in_maps.append({
            "xg_t": xg,
            "cw_pt": np.ascontiguousarray(cwg.reshape(-1, 128).T),
            "wg_t": np.ascontiguousarray(
                wg[list(es)].transpose(0, 2, 1)).astype(NP_BF16),
            "wu_t": np.ascontiguousarray(
                wu[list(es)].transpose(0, 2, 1)).astype(NP_BF16),
            "wd_t": np.ascontiguousarray(
                wd[list(es)].transpose(0, 2, 1)).astype(NP_BF16),
            "xs_t": np.ascontiguousarray(xt_bf[:, sl]),
            "xres": np.ascontiguousarray(xf[sl]),
            "ident_in": np.eye(128, dtype=NP_BF16),
            "wsg_t": wsg_bf,
            "wsu_t": wsu_bf,
            "wsd_t": wsd_bf,
        })
    return in_maps


def _combine(results, caps, pairs, idx: list[np.ndarray]) -> np.ndarray:
    out = np.empty((N, H), np.float32)
    bases = [0, caps[0]]
    for core in range(NCORES):
        out[core * NSH:(core + 1) * NSH] = results[core]["ybase"]
    for core in range(NCORES):
        ygr = np.asarray(results[core]["yg"], np.float32)
        for j, e in enumerate(pairs[core]):
            ne = len(idx[e])
            out[idx[e]] += ygr[bases[j]:bases[j] + ne]
    return out.reshape(B, S, H)


def _route(inputs: dict):
    xf = np.asarray(inputs["hidden_states"], np.float32).reshape(N, H)
    cw = _gate_cw(xf, np.asarray(inputs["gate_w"], np.float32),
                  np.asarray(inputs["gate_bias"], np.float32))
    idx = [np.nonzero(cw[:, e])[0] for e in range(E)]
    loads = np.array([len(i) for i in idx])
    order = np.argsort(-loads, kind="stable")
    bigs, smalls = order[:NCORES], order[NCORES:][::-1]
    pairs = [(int(a), int(b)) for a, b in zip(bigs, smalls)]
    cea = max(MIN_CE, -(-int(loads[bigs].max()) // 128) * 128)
    ceb = max(1024, -(-int(loads[smalls].max()) // 128) * 128)
    return cw, idx, (cea, ceb), pairs


def _run(inputs: dict, trace: bool = False, tmpdir: str | None = None):
    cw, idx, caps, pairs = _route(inputs)
    nc = _build(*caps)
    in_maps = _prepare(inputs, caps, pairs, cw, idx)
    res = run_bass_kernel_spmd(nc, in_maps, list(range(NCORES)),
                               trace=trace, tmpdir=tmpdir)
    return _combine(res.results, caps, pairs, idx), res


def kernel(**inputs) -> np.ndarray:
    out, _ = _run(inputs, trace=False)
    return out


def _install_prof_shim():
    """Make run_bass_kernel_spmd(trace=True) work under axon in this image."""
    if "antenv.axon_hooks" in sys.modules:
        return
    try:
        from trn_agent_boot.trn_boot import _ntff_profile_via_ctypes
        hook = _ntff_profile_via_ctypes("/opt/axon/libaxon_pjrt.so")
    except Exception:
        hook = None
    mod = types.ModuleType("antenv.axon_hooks")
    mod.get_axon_ntff_profile_hook = lambda: hook
    mod.set_axon_ntff_profile_hook = lambda h: None
    sys.modules["antenv.axon_hooks"] = mod
    import concourse.bass_utils as bu
    bu.upload_artifacts = lambda tmpdir: tmpdir


def kernel_traced(tmpdir=None, all_cores=False, **inputs):
    """Returns (output, BassKernelResults with exec_time_ns)."""
    _install_prof_shim()
    if all_cores:
        os.environ["BASS_PERFETTO_PROFILE_ALL_CORES"] = "1"
    out, res = _run(inputs, trace=True, tmpdir=tmpdir)
    return out, res
